# revision 5
# baseline (speedup 1.0000x reference)
"""Maxwell rheological model kernel for Trainium2 (8 NeuronCores, SPMD).

Recurrence per batch row (a = E/ETA = 2, E_INFTY = 1, E = 2):
    gamma[0] = 0
    gamma[n+1] = (1 - 2*dt[n]) * gamma[n] + 2*dt[n] * eps[n]
    sigma[n+1] = 3*eps[n+1] - 2*gamma[n+1];  sigma[0] = 0

fp16 wire format (inputs quantized to fp16 on host, output returned as
fp16 and scaled back on host): halves HBM traffic vs f32, which is the
bottleneck for this memory-bound problem. Verified max rel err 1.6e-2
vs the f32 reference (gate 2e-2), dominated by input quantization.

On-chip math per core (all fp16 tiles, scan carry is fp32 internal),
spread across engines so the DVE only runs the (serial) scan + one
aligned 2x-mode subtract:
    C[m] = 1 - 2*dt[m]     (ACT; C=0 at row starts -> scan resets)
    D[m] = dt[m]*eps[m]    (GPSIMD tensor_tensor)
    Z    = scan(C, D)      (DVE tensor_tensor_scan, z = gamma/2)
    E75s[m] = 0.75*eps[m+1](ACT, absorbs the odd offset)
    S[m] = E75s[m] - Z[m]  (DVE tensor_tensor, 2x mode) = sigma[m+1]/4
Host multiplies by 4 and zeroes column 0.

Layout trick: the per-core [2048, 2048] shard is viewed as [128, 16*2048]
(partition p owns 16 consecutive DRAM rows), so every DMA moves
contiguous multi-KB lines per partition. Rows are concatenated along the
free dim; the scan crosses row boundaries but C=0 there resets the
recurrence exactly.

Batch is sharded across 8 cores (data parallel, no collectives).
"""

import sys

if "/opt/trn_rl_repo" not in sys.path:
    sys.path.insert(0, "/opt/trn_rl_repo")

import numpy as np

import concourse.bacc as bacc
import concourse.mybir as mybir
from concourse.bass_utils import run_bass_kernel_spmd
from concourse.tile import TileContext

B, T = 16384, 2048
N_CORES = 8
B_CORE = B // N_CORES          # 2048 rows per core
P = 128
ROWS_PER_PART = B_CORE // P    # 16 DRAM rows per partition
R = 2                          # rows per chunk
N = R * T                      # free-dim elements per chunk
N_CHUNKS = ROWS_PER_PART // R
W = ROWS_PER_PART * T          # 32768 free-dim elements per partition

_prog = None


def _build():
    f16 = mybir.dt.float16
    Alu = mybir.AluOpType
    Act = mybir.ActivationFunctionType
    nc = bacc.Bacc(
        "TRN2",
        target_bir_lowering=False,
        debug=False,
        enable_asserts=False,
    )
    strains = nc.dram_tensor("strains", [P, W], f16, kind="ExternalInput").ap()
    dts = nc.dram_tensor("dts", [P, W], f16, kind="ExternalInput").ap()
    out = nc.dram_tensor("out", [P, W], f16, kind="ExternalOutput").ap()
    with TileContext(nc) as tc:
        with (
            tc.tile_pool(name="pin", bufs=3) as pin,
            tc.tile_pool(name="pact", bufs=3) as pact,
            tc.tile_pool(name="pmid", bufs=2) as pmid,
            tc.tile_pool(name="pout", bufs=2) as pout,
        ):
            for k in range(N_CHUNKS):
                base = k * N
                dt_t = pin.tile([P, N], f16, tag="dt")
                ep_t = pin.tile([P, N], f16, tag="eps")
                c_t = pact.tile([P, N], f16, tag="c")
                e_t = pact.tile([P, N], f16, tag="e75")
                d_t = pmid.tile([P, N], f16, tag="d")
                z_t = pmid.tile([P, N], f16, tag="z")
                s_t = pout.tile([P, N], f16, tag="sig")

                nc.sync.dma_start(out=dt_t[:, :], in_=dts[:, base : base + N])
                nc.sync.dma_start(out=ep_t[:, :], in_=strains[:, base : base + N])

                # C[m] = 1 - 2*dt[m]; C = 0 at row starts (gamma[0] = 0).
                nc.scalar.activation(
                    out=c_t[:, 1:N],
                    in_=dt_t[:, 1:N],
                    func=Act.Copy,
                    scale=-2.0,
                    bias=1.0,
                )
                nc.vector.memset(c_t[:, 0:N:T], 0.0)

                # D[m] = dt[m] * eps[m]  (scan's data1; at row starts this is
                # the reset value gamma_1/2 = dt0*eps0).  GPSIMD keeps the
                # DVE free for the scan.
                nc.gpsimd.tensor_tensor(
                    out=d_t[:, :], in0=dt_t[:, :], in1=ep_t[:, :], op=Alu.mult
                )

                # Z = scan(C, D): z[m] = C[m]*z[m-1] + D[m]  (fp32 carry)
                nc.vector.tensor_tensor_scan(
                    out=z_t[:, :],
                    data0=c_t[:, :],
                    data1=d_t[:, :],
                    initial=0.0,
                    op0=Alu.mult,
                    op1=Alu.add,
                )

                # E75s[m] = 0.75*eps[m+1] (ACT absorbs the +1 shift so the
                # subtract below stays 4B-aligned for the DVE 2x mode).
                nc.scalar.activation(
                    out=e_t[:, 0 : N - 1],
                    in_=ep_t[:, 1:N],
                    func=Act.Copy,
                    scale=0.75,
                )

                # S[m] = E75s[m] - Z[m] = sigma[m+1]/4
                nc.vector.tensor_tensor(
                    out=s_t[:, 0 : N - 1],
                    in0=e_t[:, 0 : N - 1],
                    in1=z_t[:, 0 : N - 1],
                    op=Alu.subtract,
                )

                # Store sigma[base+1 : base+N]; column-0 positions of each row
                # are never stored (host writes the sigma[0]=0 column).
                nc.scalar.dma_start(
                    out=out[:, base + 1 : base + N], in_=s_t[:, 0 : N - 1]
                )
    nc.compile()
    return nc


def _get_prog():
    global _prog
    if _prog is None:
        _prog = _build()
    return _prog


def _run(strains, dts, **kwargs):
    nc = _get_prog()
    s16 = np.ascontiguousarray(strains, dtype=np.float16).reshape(
        N_CORES, P, W
    )
    d16 = np.ascontiguousarray(dts, dtype=np.float16).reshape(N_CORES, P, W)
    in_maps = [
        {"strains": s16[c], "dts": d16[c]} for c in range(N_CORES)
    ]
    res = run_bass_kernel_spmd(nc, in_maps, core_ids=list(range(N_CORES)), **kwargs)
    parts = [np.asarray(r["out"]).reshape(B_CORE, T) for r in res.results]
    full = np.concatenate(parts, axis=0).astype(np.float32)
    full *= 4.0
    full[:, 0] = 0.0
    return full, res


def kernel(strains, dts):
    out, _ = _run(strains, dts)
    return out


if __name__ == "__main__":
    rng = np.random.default_rng(0)
    eps = rng.standard_normal((B, T), dtype=np.float32)
    dts_a = rng.random((B, T), dtype=np.float32)
    out = kernel(eps, dts_a)
    print("ran ok", out.shape, out.dtype)


# revision 7
# speedup vs baseline: 1.2870x; 1.2870x over previous
"""Maxwell rheological model kernel for Trainium2 (8 NeuronCores, SPMD).

Recurrence per batch row (a = E/ETA = 2, E_INFTY = 1, E = 2):
    gamma[0] = 0
    gamma[n+1] = (1 - 2*dt[n]) * gamma[n] + 2*dt[n] * eps[n]
    sigma[n+1] = 3*eps[n+1] - 2*gamma[n+1];  sigma[0] = 0

fp16 wire format (inputs quantized to fp16 on host, output returned as
fp16 and scaled back on host): halves HBM traffic vs f32, which is the
bottleneck for this memory-bound problem. Verified max rel err 1.6e-2
vs the f32 reference (gate 2e-2), dominated by input quantization.

On-chip math per core (all fp16 tiles, scan carry is fp32 internal):
    C[m] = 1 - 2*dt[m]     (ACT; C=0 at row starts -> scan resets)
    D[m] = dt[m]*eps[m]    (DVE tensor_tensor, 2x mode)
    Z    = scan(C, D)      (DVE tensor_tensor_scan, z = gamma/2)
    E75s[m] = 0.75*eps[m+1](ACT, absorbs the odd offset)
    S[m] = E75s[m] - Z[m]  (DVE tensor_tensor, 2x mode) = sigma[m+1]/4
Host multiplies by 4 and zeroes column 0.

Layout trick: the per-core [2048, 2048] shard is viewed as [128, 16*2048]
(partition p owns 16 consecutive DRAM rows), so every DMA moves
contiguous multi-KB lines per partition. Rows are concatenated along the
free dim; the scan crosses row boundaries but C=0 there resets the
recurrence exactly. First/last chunks are processed in column segments
(chained scans) to shorten the pipeline head and tail.

Batch is sharded across 8 cores (data parallel, no collectives).
"""

import sys

if "/opt/trn_rl_repo" not in sys.path:
    sys.path.insert(0, "/opt/trn_rl_repo")

import numpy as np

import concourse.bacc as bacc
import concourse.mybir as mybir
from concourse.bass_utils import run_bass_kernel_spmd
from concourse.tile import TileContext

B, T = 16384, 2048
N_CORES = 8
B_CORE = B // N_CORES          # 2048 rows per core
P = 128
ROWS_PER_PART = B_CORE // P    # 16 DRAM rows per partition
R = 4                          # rows per chunk
N = R * T                      # free-dim elements per chunk
N_CHUNKS = ROWS_PER_PART // R
W = ROWS_PER_PART * T          # 32768 free-dim elements per partition

_prog = None


def _build():
    f16 = mybir.dt.float16
    Alu = mybir.AluOpType
    Act = mybir.ActivationFunctionType
    nc = bacc.Bacc(
        "TRN2",
        target_bir_lowering=False,
        debug=False,
        enable_asserts=False,
    )
    strains = nc.dram_tensor("strains", [P, W], f16, kind="ExternalInput").ap()
    dts = nc.dram_tensor("dts", [P, W], f16, kind="ExternalInput").ap()
    out = nc.dram_tensor("out", [P, W], f16, kind="ExternalOutput").ap()
    with TileContext(nc) as tc:
        with (
            tc.tile_pool(name="pin", bufs=2) as pin,
            tc.tile_pool(name="pact", bufs=2) as pact,
            tc.tile_pool(name="pmid", bufs=1) as pmid,
            tc.tile_pool(name="pout", bufs=2) as pout,
        ):
            for k in range(N_CHUNKS):
                base = k * N
                dt_t = pin.tile([P, N], f16, tag="dt")
                ep_t = pin.tile([P, N], f16, tag="eps")
                c_t = pact.tile([P, N], f16, tag="c")
                e_t = pact.tile([P, N], f16, tag="e75")
                d_t = pmid.tile([P, N], f16, tag="d")
                z_t = pmid.tile([P, N], f16, tag="z")
                s_t = pout.tile([P, N], f16, tag="sig")

                # First chunk: segment the loads + compute (chained scans) so
                # the DVE starts ~10us earlier. T-sized segments align with
                # row boundaries, so every segment scan starts with C=0 and a
                # 0.0 initial is exact.
                if k == 0:
                    bounds = [0, T, N]
                else:
                    bounds = [0, N]
                segs = list(zip(bounds[:-1], bounds[1:]))

                for lo, hi in segs:
                    nc.sync.dma_start(
                        out=dt_t[:, lo:hi], in_=dts[:, base + lo : base + hi]
                    )
                    nc.sync.dma_start(
                        out=ep_t[:, lo:hi], in_=strains[:, base + lo : base + hi]
                    )

                for lo, hi in segs:
                    # C[m] = 1 - 2*dt[m]; C = 0 at row starts (gamma[0] = 0).
                    nc.scalar.activation(
                        out=c_t[:, lo + 1 : hi],
                        in_=dt_t[:, lo + 1 : hi],
                        func=Act.Copy,
                        scale=-2.0,
                        bias=1.0,
                    )
                    nc.vector.memset(c_t[:, lo:hi:T], 0.0)

                    # D[m] = dt[m]*eps[m] (scan data1; the row-start value is
                    # the reset gamma_1/2 = dt0*eps0).
                    nc.vector.tensor_tensor(
                        out=d_t[:, lo:hi],
                        in0=dt_t[:, lo:hi],
                        in1=ep_t[:, lo:hi],
                        op=Alu.mult,
                    )

                    # Z = scan(C, D): z[m] = C[m]*z[m-1] + D[m] (fp32 carry).
                    # Segments start at row boundaries: initial is moot.
                    nc.vector.tensor_tensor_scan(
                        out=z_t[:, lo:hi],
                        data0=c_t[:, lo:hi],
                        data1=d_t[:, lo:hi],
                        initial=0.0,
                        op0=Alu.mult,
                        op1=Alu.add,
                    )

                    # E75s[m] = 0.75*eps[m+1] (ACT absorbs the +1 shift so the
                    # subtract stays 4B-aligned for the DVE 2x mode).
                    ch = min(hi, N - 1)
                    nc.scalar.activation(
                        out=e_t[:, lo:ch],
                        in_=ep_t[:, lo + 1 : ch + 1],
                        func=Act.Copy,
                        scale=0.75,
                    )

                    # S[m] = E75s[m] - Z[m] = sigma[m+1]/4
                    nc.vector.tensor_tensor(
                        out=s_t[:, lo:ch],
                        in0=e_t[:, lo:ch],
                        in1=z_t[:, lo:ch],
                        op=Alu.subtract,
                    )

                    # Store sigma[base+lo+1 : base+ch+1]; column-0 positions of
                    # each row are never stored (host writes sigma[0]=0).
                    nc.scalar.dma_start(
                        out=out[:, base + lo + 1 : base + ch + 1],
                        in_=s_t[:, lo:ch],
                    )
    nc.compile()
    return nc


def _get_prog():
    global _prog
    if _prog is None:
        _prog = _build()
    return _prog


def _run(strains, dts, **kwargs):
    nc = _get_prog()
    s16 = np.ascontiguousarray(strains, dtype=np.float16).reshape(
        N_CORES, P, W
    )
    d16 = np.ascontiguousarray(dts, dtype=np.float16).reshape(N_CORES, P, W)
    in_maps = [
        {"strains": s16[c], "dts": d16[c]} for c in range(N_CORES)
    ]
    res = run_bass_kernel_spmd(nc, in_maps, core_ids=list(range(N_CORES)), **kwargs)
    parts = [np.asarray(r["out"]).reshape(B_CORE, T) for r in res.results]
    full = np.concatenate(parts, axis=0).astype(np.float32)
    full *= 4.0
    full[:, 0] = 0.0
    return full, res


def kernel(strains, dts):
    out, _ = _run(strains, dts)
    return out


if __name__ == "__main__":
    rng = np.random.default_rng(0)
    eps = rng.standard_normal((B, T), dtype=np.float32)
    dts_a = rng.random((B, T), dtype=np.float32)
    out = kernel(eps, dts_a)
    print("ran ok", out.shape, out.dtype)
